# revision 33
# baseline (speedup 1.0000x reference)
"""MoE top-1 routing kernel for Trainium2 (8 NeuronCores, expert-parallel).

Strategy:
  - Gate (x @ Wg.T + bg, argmax) on host in float64 (exact vs fp32 ref:
    min top-2 gap ~1.2e-5 >> fp32 rounding noise).
  - Tokens grouped by expert on host (the all-to-all dispatch); core e runs
    the dense SwiGLU FFN for expert e's tokens (capacity-padded to C).
  - Combine on host (top-1 => weight 1.0); outputs come back bf16 and are
    upcast to f32.

Device kernel: all matmuls are fp8(e4m3) DoubleRow (two 128-contraction
k-tiles per matmul, 0.5 PE cycles/output-row — 2x the bf16/f32r rate).
Plain fp8 would cost ~5% accuracy, so operands are hi/lo error-compensated:

  A ~= fp8(s*A) [hi] + fp8(16*(s*A - hi))/16 [lo]   (residual ~0.16%)

and each 128-contraction product uses 3 DoubleRow terms:
  W*x  ~= Whi*xhi + Whi*xlo_s + Wlo*xhi_s    (stage A, h1 and h2)
  W3*g ~= W3hi*ghi + W3hi*glo + W3lo_s*ghi   (stage B)
with *_s = fp8(arr/16) (exact exponent shift, host-precomputed), and on
device ghi = fp8(p), glo = fp8(p - ghi) from the f32 product p = sil*ps2.
End-to-end rel-l2 vs the f32 reference: ~2.5e-3 (gate is 2e-2).

Scales (powers of 2, exact): x*8, W1*128, W2*4, W3*128, so ps1 = 1024*h1
(silu applied with input-scale 2^-10), ps2 = 32*h2, p = 32*g (|p|max ~160
< fp8 max 240), stage-B accum = 4096*y (rescaled to bf16 on copy-out).

PE cost: 54 cycles/token (36 stage A + 18 stage B) vs 72 for bf16/f32r.
Chunk c's stage-B halves are emitted between chunk c+1's stage-A pairs so
the PE doesn't wait for the ACT/DVE epilogue chain (silu -> p -> ghi/glo);
chunk 0 is emitted term-class-outer to match the DMA preload arrival order.
"""

import sys
from contextlib import ExitStack

if "/opt/trn_rl_repo" not in sys.path:
    sys.path.insert(0, "/opt/trn_rl_repo")

import numpy as np

P = 128
D = 768          # model dim
E = 8            # experts == cores
F = 469          # ffn hidden
FP = 512         # F padded to a multiple of 128
KP = 3           # k-PAIRS over D (6 k-tiles of 128)
FPAIR = 2        # f-pairs for stage-B contraction
DT = D // P      # 6 out-tiles over D
MIN_C = 128      # capacity floor
CHUNK = 512      # tokens per chunk (PSUM bank = 512 f32)

SX = 8.0         # x quant scale (2^3)
SW1 = 128.0      # W1 quant scale (2^7)
SW2 = 4.0        # W2 quant scale (2^2): p = 32*g stays under fp8 max 240
SW3 = 128.0      # W3 quant scale (2^7)
SILU_SCALE = 1.0 / (SX * SW1)        # 2^-10
OUT_SCALE = 1.0 / (SW3 * SX * SW2)   # 2^-12

_cache = {}


def _chunks(C):
    """[512]*k + small even tail (small tail keeps the pipeline drain short)."""
    sizes = []
    left = C
    while left > CHUNK:
        sizes.append(CHUNK)
        left -= CHUNK
    sizes.append(left)
    assert all(s % 2 == 0 for s in sizes) and sum(sizes) == C
    out = []
    off = 0
    for s in sizes:
        out.append((off, s))
        off += s
    return out


def _build(C):
    import concourse.bacc as bacc
    import concourse.tile as tile
    from concourse import mybir

    f32 = mybir.dt.float32
    f8 = mybir.dt.float8e4
    bf16 = mybir.dt.bfloat16
    DR = mybir.MatmulPerfMode.DoubleRow
    silu = mybir.ActivationFunctionType.Silu

    nc = bacc.Bacc("TRN2", target_bir_lowering=False, debug=False, num_devices=E)

    # DRAM inputs, host-packed fp8.
    # xq: variant axis v = (xhi, xlo_s, xhi_s); w12: v = (w1hi, w2hi, w1lo, w2lo);
    # w3: v = (w3hi, w3lo_s). Inner layout [P, 2, cols] = (partition, DR k-slot, col).
    xq_d = nc.dram_tensor("xq", [KP, P, 3, 2, C], f8, kind="ExternalInput").ap()
    w12_d = nc.dram_tensor("w12", [KP, P, 4, 2, FP], f8, kind="ExternalInput").ap()
    w3_d = nc.dram_tensor("w3", [FPAIR, P, 2, 2, D], f8, kind="ExternalInput").ap()
    yt_d = nc.dram_tensor("yt", [DT, P, C], bf16, kind="ExternalOutput").ap()

    chunks = _chunks(C)
    nch = len(chunks)

    with tile.TileContext(nc) as tc, ExitStack() as ctx:
        wpool = ctx.enter_context(tc.tile_pool(name="w", bufs=1))
        spool = ctx.enter_context(tc.tile_pool(name="s", bufs=4))
        ppool = ctx.enter_context(tc.tile_pool(name="p", bufs=4))
        gpool = ctx.enter_context(tc.tile_pool(name="g", bufs=3))
        opool = ctx.enter_context(tc.tile_pool(name="o", bufs=4))
        pspool = ctx.enter_context(tc.tile_pool(name="ps", bufs=8, space="PSUM"))

        xq = [wpool.tile([P, 3, 2, C], f8, tag=f"xq{k}", name=f"xq{k}")
              for k in range(KP)]
        w12 = [wpool.tile([P, 4, 2, FP], f8, tag=f"w12{k}", name=f"w12{k}")
               for k in range(KP)]
        w3 = [wpool.tile([P, 2, 2, D], f8, tag=f"w3{k}", name=f"w3{k}")
              for k in range(FPAIR)]

        def xdma(ci):
            n0, nn = chunks[ci]
            for kp in range(KP):
                nc.sync.dma_start(
                    xq[kp][:, :, :, n0:n0 + nn],
                    xq_d[kp, :, :, :, n0:n0 + nn],
                )

        # ---- preload, ordered to match chunk-0 consumption order.
        # v-axis of w12 is (w1hi, w2hi, w1lo, w2lo): hi planes load first
        # (one DMA), lo planes (hi_s term class) later.
        nn_0 = chunks[0][1]
        for kp in range(KP):
            nc.sync.dma_start(w12[kp][:, 0:2], w12_d[kp, :, 0:2])   # w1hi, w2hi
            nc.sync.dma_start(xq[kp][:, :, :, :nn_0], xq_d[kp, :, :, :, :nn_0])
        for kp in range(KP):
            nc.sync.dma_start(w12[kp][:, 2:4], w12_d[kp, :, 2:4])   # w1lo, w2lo
        if nch > 1:
            xdma(1)
        for fp2 in range(FPAIR):
            nc.sync.dma_start(w3[fp2][:], w3_d[fp2])
        for ci in range(2, nch):
            xdma(ci)

        def msl(m):
            return slice(m * P, (m + 1) * P)

        def stage_a_chunk0():
            """All 8 accumulators, term-class-outer (matches DMA arrival)."""
            n0, nn = chunks[0]
            csl = slice(n0, n0 + nn)
            ps1 = {m: pspool.tile([P, nn], f32, tag="ps", name=f"ps1_0_{m}")
                   for m in range(4)}
            ps2 = {m: pspool.tile([P, nn], f32, tag="ps", name=f"ps2_0_{m}")
                   for m in range(4)}
            for cls, xv in enumerate((0, 1, 2)):  # hi, lo_s, hi_s
                for psd, wv in ((ps1, 0 if cls < 2 else 2), (ps2, 1 if cls < 2 else 3)):
                    for kp in range(KP):
                        for m in range(4):
                            nc.tensor.matmul(
                                psd[m][:], w12[kp][:, wv, :, msl(m)],
                                xq[kp][:, xv, :, csl],
                                start=(cls == 0 and kp == 0),
                                stop=(cls == 2 and kp == KP - 1), perf_mode=DR,
                            )
            return ps1, ps2

        def _accum(ci, m, psd_tile, wh, wl):
            """The 9 DR matmuls of one accumulator (3 term classes x 3 kp)."""
            n0, nn = chunks[ci]
            csl = slice(n0, n0 + nn)
            for xv, wv, first, last in ((0, wh, True, False),
                                        (1, wh, False, False),
                                        (2, wl, False, True)):
                for kp in range(KP):
                    nc.tensor.matmul(
                        psd_tile[:], w12[kp][:, wv, :, msl(m)],
                        xq[kp][:, xv, :, csl],
                        start=(first and kp == 0),
                        stop=(last and kp == KP - 1), perf_mode=DR,
                    )

        def epilogue_pair(ci, ms, ps, sils, g):
            """Breadth-first: DVE p(a), p(b); ACT ghi(a), ghi(b);
            DVE glo(a), glo(b). Work split evenly between ACT and DVE."""
            nn = chunks[ci][1]
            pt = {}
            for m in ms:
                p = ppool.tile([P, nn], f32, tag="p", name=f"p{ci}_{m}")
                nc.vector.tensor_mul(p[:], sils[m][:], ps[m][1][:])
                pt[m] = p
            for m in ms:
                gh, _ = g[m // 2]
                nc.scalar.copy(gh[:, m % 2, :], pt[m][:])     # ghi = fp8(p)
            for m in ms:
                gh, gl = g[m // 2]
                nc.vector.tensor_sub(gl[:, m % 2, :], pt[m][:], gh[:, m % 2, :])

        def stage_a_pair(ci, mp, g):
            """Per-m: ps1 mms, silu (ACT, fires as soon as ps1 closes),
            ps2 mms; then the pair's breadth-first DVE epilogue."""
            nn = chunks[ci][1]
            ms = (2 * mp, 2 * mp + 1)
            ps, sils = {}, {}
            for m in ms:
                ps1 = pspool.tile([P, nn], f32, tag="ps", name=f"ps1_{ci}_{m}")
                _accum(ci, m, ps1, 0, 2)
                sil = spool.tile([P, nn], f32, tag="sil", name=f"sil{ci}_{m}")
                nc.scalar.activation(sil[:], ps1[:], silu, scale=SILU_SCALE)
                ps2 = pspool.tile([P, nn], f32, tag="ps", name=f"ps2_{ci}_{m}")
                _accum(ci, m, ps2, 1, 3)
                ps[m] = (ps1, ps2)
                sils[m] = sil
            epilogue_pair(ci, ms, ps, sils, g)

        def _b_out(ci, d, pso, ots):
            n0, nn = chunks[ci]
            t, r = divmod(d, 3)
            if r == 0:
                ots[t] = opool.tile([P, 3, nn], bf16, tag="ot", name=f"ot{ci}_{t}")
            if r % 2 == 0:
                nc.scalar.mul(ots[t][:, r, :], pso[:], OUT_SCALE)
            else:
                nc.vector.tensor_scalar_mul(ots[t][:, r, :], pso[:], OUT_SCALE)
            if r == 2:
                (nc.sync if t == 0 else nc.scalar).dma_start(
                    yt_d[3 * t:3 * t + 3, :, n0:n0 + nn].rearrange("j p c -> p j c"),
                    ots[t][:],
                )

        def stage_b_half(ci, half, g, ots):
            """d-tiles [3*half, 3*half+3): 6 DR matmuls each + scaled copy + store."""
            n0, nn = chunks[ci]
            for d in range(3 * half, 3 * half + 3):
                pso = pspool.tile([P, nn], f32, tag="ps", name=f"pso{ci}_{d}")
                for fp2 in range(FPAIR):
                    gh, gl = g[fp2]
                    nc.tensor.matmul(pso[:], w3[fp2][:, 0, :, msl(d)], gh[:],
                                     start=(fp2 == 0), stop=False, perf_mode=DR)
                    nc.tensor.matmul(pso[:], w3[fp2][:, 0, :, msl(d)], gl[:],
                                     start=False, stop=False, perf_mode=DR)
                    nc.tensor.matmul(pso[:], w3[fp2][:, 1, :, msl(d)], gh[:],
                                     start=False, stop=(fp2 == FPAIR - 1),
                                     perf_mode=DR)
                _b_out(ci, d, pso, ots)

        def stage_b_fp_outer(ci, g, ots):
            """Final-chunk stage B: all fpair-0 terms first so the PE only
            waits on the first two epilogues, then fpair-1 + copy/store."""
            n0, nn = chunks[ci]
            psos = {d: pspool.tile([P, nn], f32, tag="ps", name=f"pso{ci}_{d}")
                    for d in range(DT)}
            for d in range(DT):
                gh, gl = g[0]
                nc.tensor.matmul(psos[d][:], w3[0][:, 0, :, msl(d)], gh[:],
                                 start=True, stop=False, perf_mode=DR)
                nc.tensor.matmul(psos[d][:], w3[0][:, 0, :, msl(d)], gl[:],
                                 start=False, stop=False, perf_mode=DR)
                nc.tensor.matmul(psos[d][:], w3[0][:, 1, :, msl(d)], gh[:],
                                 start=False, stop=False, perf_mode=DR)
            for d in range(DT):
                gh, gl = g[1]
                nc.tensor.matmul(psos[d][:], w3[1][:, 0, :, msl(d)], gh[:],
                                 start=False, stop=False, perf_mode=DR)
                nc.tensor.matmul(psos[d][:], w3[1][:, 0, :, msl(d)], gl[:],
                                 start=False, stop=False, perf_mode=DR)
                nc.tensor.matmul(psos[d][:], w3[1][:, 1, :, msl(d)], gh[:],
                                 start=False, stop=True, perf_mode=DR)
                _b_out(ci, d, psos[d], ots)

        def gtiles(ci):
            nn = chunks[ci][1]
            return {mp: (gpool.tile([P, 2, nn], f8, tag=f"gh{mp}", name=f"gh{ci}_{mp}"),
                         gpool.tile([P, 2, nn], f8, tag=f"gl{mp}", name=f"gl{ci}_{mp}"))
                    for mp in range(2)}

        # ---- software-pipelined emission ----
        g0 = gtiles(0)
        ps1_0, ps2_0 = stage_a_chunk0()
        sils0 = {}
        for m in range(4):
            sil = spool.tile([P, nn_0], f32, tag="sil", name=f"sil0_{m}")
            nc.scalar.activation(sil[:], ps1_0[m][:], silu, scale=SILU_SCALE)
            sils0[m] = sil
        ps0 = {m: (ps1_0[m], ps2_0[m]) for m in range(4)}
        epilogue_pair(0, (0, 1), ps0, sils0, g0)
        epilogue_pair(0, (2, 3), ps0, sils0, g0)
        prev = (0, g0, {})
        for ci in range(1, nch):
            g = gtiles(ci)
            stage_a_pair(ci, 0, g)
            stage_b_half(prev[0], 0, prev[1], prev[2])
            stage_a_pair(ci, 1, g)
            stage_b_half(prev[0], 1, prev[1], prev[2])
            prev = (ci, g, {})
        stage_b_fp_outer(prev[0], prev[1], prev[2])

    nc.compile()
    return nc


LAST_RESULTS = None  # BassKernelResults of the most recent run (for test harness)


def kernel(x, Wg, bg, W1, W2, W3):
    global LAST_RESULTS
    import ml_dtypes
    from concourse.bass_utils import run_bass_kernel_spmd

    f8np = ml_dtypes.float8_e4m3

    x = np.asarray(x)
    Wg, bg = np.asarray(Wg), np.asarray(bg)
    W1, W2, W3 = np.asarray(W1), np.asarray(W2), np.asarray(W3)
    B, S, d = x.shape
    T = B * S
    assert d == D and Wg.shape == (E, D)

    xf = np.ascontiguousarray(x.reshape(T, D))

    # ---- host gate + top-1 routing (fp64: exact vs any fp32 backend) ----
    gate = xf.astype(np.float64) @ Wg.astype(np.float64).T + bg.astype(np.float64)
    eid = np.argmax(gate, axis=1)
    counts = np.bincount(eid, minlength=E)
    order = np.argsort(eid, kind="stable")
    offs = np.concatenate(([0], np.cumsum(counts)))

    C = max(MIN_C, 2 * int(-(-counts.max() // 2)))
    if C not in _cache:
        _cache[C] = _build(C)
    nc = _cache[C]

    def q8(a):
        return a.astype(f8np)

    def hi_lo(a, s):
        hi = q8(a * s)
        lo = q8((a * s - hi.astype(np.float32)) * 16.0)
        return hi, lo

    def pack(a, npair):
        # [R, cols] with R = npair*2*128 -> [npair, P, 2, cols]
        cols = a.shape[1]
        return a.reshape(npair, 2, P, cols).transpose(0, 2, 1, 3)

    # ---- per-core inputs (dispatch) ----
    in_maps = []
    tok_lists = []
    for e in range(E):
        toks = order[offs[e]:offs[e + 1]]
        tok_lists.append(toks)
        ce = len(toks)

        xT = np.zeros((D, C), dtype=np.float32)
        if ce:
            xT[:, :ce] = xf[toks].T
        xhi, xlo = hi_lo(xT, SX)
        xlos = q8(xlo.astype(np.float32) / 16.0)
        xhis = q8(xhi.astype(np.float32) / 16.0)
        xq = np.stack([pack(v, KP) for v in (xhi, xlos, xhis)], axis=2)

        w1T = np.zeros((D, FP), dtype=np.float32)
        w1T[:, :F] = W1[e].T
        w1hi, w1lo = hi_lo(w1T, SW1)
        w2T = np.zeros((D, FP), dtype=np.float32)
        w2T[:, :F] = W2[e].T
        w2hi, w2lo = hi_lo(w2T, SW2)
        w12 = np.stack([pack(v, KP) for v in (w1hi, w2hi, w1lo, w2lo)], axis=2)

        w3T = np.zeros((FP, D), dtype=np.float32)
        w3T[:F, :] = W3[e].T
        w3hi, w3lo = hi_lo(w3T, SW3)
        w3ls = q8(w3lo.astype(np.float32) / 16.0)
        w3 = np.stack([pack(v, FPAIR) for v in (w3hi, w3ls)], axis=2)

        in_maps.append({
            "xq": np.ascontiguousarray(xq),
            "w12": np.ascontiguousarray(w12),
            "w3": np.ascontiguousarray(w3),
        })

    res = run_bass_kernel_spmd(nc, in_maps, list(range(E)))
    LAST_RESULTS = res

    # ---- combine: scatter outputs back to token order ----
    y = np.empty((T, D), dtype=np.float32)
    for e in range(E):
        toks = tok_lists[e]
        if len(toks):
            yT = res.results[e]["yt"].reshape(D, C)
            y[toks] = yT[:, :len(toks)].T.astype(np.float32)
    return y.reshape(B, S, d)


# revision 34
# speedup vs baseline: 1.0107x; 1.0107x over previous
"""MoE top-1 routing kernel for Trainium2 (8 NeuronCores, expert-parallel).

Strategy:
  - Gate (x @ Wg.T + bg, argmax) on host in float64 (exact vs fp32 ref:
    min top-2 gap ~1.2e-5 >> fp32 rounding noise).
  - Tokens grouped by expert on host (the all-to-all dispatch); core e runs
    the dense SwiGLU FFN for expert e's tokens (capacity-padded to C).
  - Combine on host (top-1 => weight 1.0); outputs come back bf16 and are
    upcast to f32.

Device kernel: all matmuls are fp8(e4m3) DoubleRow (two 128-contraction
k-tiles per matmul, 0.5 PE cycles/output-row — 2x the bf16/f32r rate).
Plain fp8 would cost ~5% accuracy, so operands are hi/lo error-compensated:

  A ~= fp8(s*A) [hi] + fp8(16*(s*A - hi))/16 [lo]   (residual ~0.16%)

and each 128-contraction product uses 3 DoubleRow terms:
  W*x  ~= Whi*xhi + Whi*xlo_s + Wlo*xhi_s    (stage A, h1 and h2)
  W3*g ~= W3hi*ghi + W3hi*glo + W3lo_s*ghi   (stage B)
with *_s = fp8(arr/16) (exact exponent shift, host-precomputed), and on
device ghi = fp8(p), glo = fp8(p - ghi) from the f32 product p = sil*ps2.
End-to-end rel-l2 vs the f32 reference: ~2.5e-3 (gate is 2e-2).

Scales (powers of 2, exact): x*8, W1*128, W2*4, W3*128, so ps1 = 1024*h1
(silu applied with input-scale 2^-10), ps2 = 32*h2, p = 32*g (|p|max ~160
< fp8 max 240), stage-B accum = 4096*y (rescaled to bf16 on copy-out).

PE cost: 54 cycles/token (36 stage A + 18 stage B) vs 72 for bf16/f32r.
Chunk c's stage-B halves are emitted between chunk c+1's stage-A pairs so
the PE doesn't wait for the ACT/DVE epilogue chain (silu -> p -> ghi/glo);
chunk 0 is emitted term-class-outer to match the DMA preload arrival order.
"""

import sys
from contextlib import ExitStack

if "/opt/trn_rl_repo" not in sys.path:
    sys.path.insert(0, "/opt/trn_rl_repo")

import numpy as np

P = 128
D = 768          # model dim
E = 8            # experts == cores
F = 469          # ffn hidden
FP = 512         # F padded to a multiple of 128
KP = 3           # k-PAIRS over D (6 k-tiles of 128)
FPAIR = 2        # f-pairs for stage-B contraction
DT = D // P      # 6 out-tiles over D
MIN_C = 128      # capacity floor
CHUNK = 512      # tokens per chunk (PSUM bank = 512 f32)

SX = 8.0         # x quant scale (2^3)
SW1 = 128.0      # W1 quant scale (2^7)
SW2 = 4.0        # W2 quant scale (2^2): p = 32*g stays under fp8 max 240
SW3 = 128.0      # W3 quant scale (2^7)
SILU_SCALE = 1.0 / (SX * SW1)        # 2^-10
OUT_SCALE = 1.0 / (SW3 * SX * SW2)   # 2^-12

_cache = {}


def _chunks(C):
    """[512]*k + small even tail (small tail keeps the pipeline drain short)."""
    sizes = []
    left = C
    while left > CHUNK:
        sizes.append(CHUNK)
        left -= CHUNK
    sizes.append(left)
    assert all(s % 2 == 0 for s in sizes) and sum(sizes) == C
    out = []
    off = 0
    for s in sizes:
        out.append((off, s))
        off += s
    return out


def _build(C):
    import concourse.bacc as bacc
    import concourse.tile as tile
    from concourse import mybir

    f32 = mybir.dt.float32
    f8 = mybir.dt.float8e4
    bf16 = mybir.dt.bfloat16
    DR = mybir.MatmulPerfMode.DoubleRow
    silu = mybir.ActivationFunctionType.Silu

    nc = bacc.Bacc("TRN2", target_bir_lowering=False, debug=False, num_devices=E)

    # DRAM inputs, host-packed fp8.
    # xq: variant axis v = (xhi, xlo_s, xhi_s); w12: v = (w1hi, w2hi, w1lo, w2lo);
    # w3: v = (w3hi, w3lo_s). Inner layout [P, 2, cols] = (partition, DR k-slot, col).
    xq_d = nc.dram_tensor("xq", [KP, P, 3, 2, C], f8, kind="ExternalInput").ap()
    w12_d = nc.dram_tensor("w12", [KP, P, 4, 2, FP], f8, kind="ExternalInput").ap()
    w3_d = nc.dram_tensor("w3", [FPAIR, P, 2, 2, D], f8, kind="ExternalInput").ap()
    yt_d = nc.dram_tensor("yt", [DT, P, C], bf16, kind="ExternalOutput").ap()

    chunks = _chunks(C)
    nch = len(chunks)

    with tile.TileContext(nc) as tc, ExitStack() as ctx:
        wpool = ctx.enter_context(tc.tile_pool(name="w", bufs=1))
        spool = ctx.enter_context(tc.tile_pool(name="s", bufs=4))
        ppool = ctx.enter_context(tc.tile_pool(name="p", bufs=4))
        gpool = ctx.enter_context(tc.tile_pool(name="g", bufs=3))
        opool = ctx.enter_context(tc.tile_pool(name="o", bufs=4))
        pspool = ctx.enter_context(tc.tile_pool(name="ps", bufs=8, space="PSUM"))

        xq = [wpool.tile([P, 3, 2, C], f8, tag=f"xq{k}", name=f"xq{k}")
              for k in range(KP)]
        w12 = [wpool.tile([P, 4, 2, FP], f8, tag=f"w12{k}", name=f"w12{k}")
               for k in range(KP)]
        w3 = [wpool.tile([P, 2, 2, D], f8, tag=f"w3{k}", name=f"w3{k}")
              for k in range(FPAIR)]

        def xdma(ci):
            n0, nn = chunks[ci]
            for kp in range(KP):
                nc.sync.dma_start(
                    xq[kp][:, :, :, n0:n0 + nn],
                    xq_d[kp, :, :, :, n0:n0 + nn],
                )

        # ---- preload, ordered to match chunk-0 consumption order.
        # v-axis of w12 is (w1hi, w2hi, w1lo, w2lo): hi planes load first
        # (one DMA), lo planes (hi_s term class) later.
        nn_0 = chunks[0][1]
        for kp in range(KP):
            nc.sync.dma_start(w12[kp][:, 0:2], w12_d[kp, :, 0:2])   # w1hi, w2hi
            nc.sync.dma_start(xq[kp][:, :, :, :nn_0], xq_d[kp, :, :, :, :nn_0])
        for kp in range(KP):
            nc.sync.dma_start(w12[kp][:, 2:4], w12_d[kp, :, 2:4])   # w1lo, w2lo
        if nch > 1:
            xdma(1)
        for fp2 in range(FPAIR):
            nc.sync.dma_start(w3[fp2][:], w3_d[fp2])
        for ci in range(2, nch):
            xdma(ci)

        def msl(m):
            return slice(m * P, (m + 1) * P)

        def stage_a_chunk0():
            """All 8 accumulators, term-class-outer (matches DMA arrival)."""
            n0, nn = chunks[0]
            csl = slice(n0, n0 + nn)
            ps1 = {m: pspool.tile([P, nn], f32, tag="ps", name=f"ps1_0_{m}")
                   for m in range(4)}
            ps2 = {m: pspool.tile([P, nn], f32, tag="ps", name=f"ps2_0_{m}")
                   for m in range(4)}
            for cls, xv in enumerate((0, 1, 2)):  # hi, lo_s, hi_s
                for psd, wv in ((ps1, 0 if cls < 2 else 2), (ps2, 1 if cls < 2 else 3)):
                    for kp in range(KP):
                        for m in range(4):
                            nc.tensor.matmul(
                                psd[m][:], w12[kp][:, wv, :, msl(m)],
                                xq[kp][:, xv, :, csl],
                                start=(cls == 0 and kp == 0),
                                stop=(cls == 2 and kp == KP - 1), perf_mode=DR,
                            )
            return ps1, ps2

        def _accum(ci, m, psd_tile, wh, wl):
            """The 9 DR matmuls of one accumulator (3 term classes x 3 kp)."""
            n0, nn = chunks[ci]
            csl = slice(n0, n0 + nn)
            for xv, wv, first, last in ((0, wh, True, False),
                                        (1, wh, False, False),
                                        (2, wl, False, True)):
                for kp in range(KP):
                    nc.tensor.matmul(
                        psd_tile[:], w12[kp][:, wv, :, msl(m)],
                        xq[kp][:, xv, :, csl],
                        start=(first and kp == 0),
                        stop=(last and kp == KP - 1), perf_mode=DR,
                    )

        def epilogue_pair(ci, ms, ps, sils, g):
            """Breadth-first: DVE p(a), p(b); ACT ghi(a), ghi(b);
            DVE glo(a), glo(b). Work split evenly between ACT and DVE."""
            nn = chunks[ci][1]
            pt = {}
            for m in ms:
                p = ppool.tile([P, nn], f32, tag="p", name=f"p{ci}_{m}")
                nc.vector.tensor_mul(p[:], sils[m][:], ps[m][1][:])
                pt[m] = p
            for m in ms:
                gh, _ = g[m // 2]
                nc.scalar.copy(gh[:, m % 2, :], pt[m][:])     # ghi = fp8(p)
            for m in ms:
                gh, gl = g[m // 2]
                nc.vector.tensor_sub(gl[:, m % 2, :], pt[m][:], gh[:, m % 2, :])

        def stage_a_pair(ci, mp, g):
            """Per-m: ps1 mms, silu (ACT, fires as soon as ps1 closes),
            ps2 mms; then the pair's breadth-first DVE epilogue."""
            nn = chunks[ci][1]
            ms = (2 * mp, 2 * mp + 1)
            ps, sils = {}, {}
            for m in ms:
                ps1 = pspool.tile([P, nn], f32, tag="ps", name=f"ps1_{ci}_{m}")
                _accum(ci, m, ps1, 0, 2)
                sil = spool.tile([P, nn], f32, tag="sil", name=f"sil{ci}_{m}")
                nc.scalar.activation(sil[:], ps1[:], silu, scale=SILU_SCALE)
                ps2 = pspool.tile([P, nn], f32, tag="ps", name=f"ps2_{ci}_{m}")
                _accum(ci, m, ps2, 1, 3)
                ps[m] = (ps1, ps2)
                sils[m] = sil
            epilogue_pair(ci, ms, ps, sils, g)

        def _b_out(ci, d, pso, ots):
            n0, nn = chunks[ci]
            q, r = divmod(d, 2)
            if r == 0:
                ot = opool.tile([P, 2, nn], bf16, tag="ot", name=f"ot{ci}_{q}")
                ots[q] = ot
                nc.scalar.mul(ot[:, 0, :], pso[:], OUT_SCALE)
            else:
                ot = ots[q]  # slot 0 filled by d-1 (possibly in half 0)
                nc.vector.tensor_scalar_mul(ot[:, 1, :], pso[:], OUT_SCALE)
                eng = (nc.sync, nc.scalar, nc.sync)[q]
                eng.dma_start(
                    yt_d[2 * q:2 * q + 2, :, n0:n0 + nn].rearrange("j p c -> p j c"),
                    ot[:],
                )

        def stage_b_half(ci, half, g, ots):
            """d-tiles [3*half, 3*half+3): 6 DR matmuls each + scaled copy + store."""
            n0, nn = chunks[ci]
            for d in range(3 * half, 3 * half + 3):
                pso = pspool.tile([P, nn], f32, tag="ps", name=f"pso{ci}_{d}")
                for fp2 in range(FPAIR):
                    gh, gl = g[fp2]
                    nc.tensor.matmul(pso[:], w3[fp2][:, 0, :, msl(d)], gh[:],
                                     start=(fp2 == 0), stop=False, perf_mode=DR)
                    nc.tensor.matmul(pso[:], w3[fp2][:, 0, :, msl(d)], gl[:],
                                     start=False, stop=False, perf_mode=DR)
                    nc.tensor.matmul(pso[:], w3[fp2][:, 1, :, msl(d)], gh[:],
                                     start=False, stop=(fp2 == FPAIR - 1),
                                     perf_mode=DR)
                _b_out(ci, d, pso, ots)

        def stage_b_fp_outer(ci, g, ots):
            """Final-chunk stage B: all fpair-0 terms first so the PE only
            waits on the first two epilogues, then fpair-1 + copy/store."""
            n0, nn = chunks[ci]
            psos = {d: pspool.tile([P, nn], f32, tag="ps", name=f"pso{ci}_{d}")
                    for d in range(DT)}
            for d in range(DT):
                gh, gl = g[0]
                nc.tensor.matmul(psos[d][:], w3[0][:, 0, :, msl(d)], gh[:],
                                 start=True, stop=False, perf_mode=DR)
                nc.tensor.matmul(psos[d][:], w3[0][:, 0, :, msl(d)], gl[:],
                                 start=False, stop=False, perf_mode=DR)
                nc.tensor.matmul(psos[d][:], w3[0][:, 1, :, msl(d)], gh[:],
                                 start=False, stop=False, perf_mode=DR)
            for d in range(DT):
                gh, gl = g[1]
                nc.tensor.matmul(psos[d][:], w3[1][:, 0, :, msl(d)], gh[:],
                                 start=False, stop=False, perf_mode=DR)
                nc.tensor.matmul(psos[d][:], w3[1][:, 0, :, msl(d)], gl[:],
                                 start=False, stop=False, perf_mode=DR)
                nc.tensor.matmul(psos[d][:], w3[1][:, 1, :, msl(d)], gh[:],
                                 start=False, stop=True, perf_mode=DR)
                # final chunk: two stores of 3 d-tiles each, one per DMA queue,
                # so both issue in parallel and the drain ends one store sooner
                t, r = divmod(d, 3)
                if r == 0:
                    ots[t] = opool.tile([P, 3, nn], bf16, tag="otf", name=f"otf{ci}_{t}")
                if d % 2 == 0:
                    nc.scalar.mul(ots[t][:, r, :], psos[d][:], OUT_SCALE)
                else:
                    nc.vector.tensor_scalar_mul(ots[t][:, r, :], psos[d][:], OUT_SCALE)
                if r == 2:
                    (nc.sync if t == 0 else nc.scalar).dma_start(
                        yt_d[3 * t:3 * t + 3, :, n0:n0 + nn].rearrange("j p c -> p j c"),
                        ots[t][:],
                    )

        def gtiles(ci):
            nn = chunks[ci][1]
            return {mp: (gpool.tile([P, 2, nn], f8, tag=f"gh{mp}", name=f"gh{ci}_{mp}"),
                         gpool.tile([P, 2, nn], f8, tag=f"gl{mp}", name=f"gl{ci}_{mp}"))
                    for mp in range(2)}

        # ---- software-pipelined emission ----
        g0 = gtiles(0)
        ps1_0, ps2_0 = stage_a_chunk0()
        sils0 = {}
        for m in range(4):
            sil = spool.tile([P, nn_0], f32, tag="sil", name=f"sil0_{m}")
            nc.scalar.activation(sil[:], ps1_0[m][:], silu, scale=SILU_SCALE)
            sils0[m] = sil
        ps0 = {m: (ps1_0[m], ps2_0[m]) for m in range(4)}
        epilogue_pair(0, (0, 1), ps0, sils0, g0)
        epilogue_pair(0, (2, 3), ps0, sils0, g0)
        prev = (0, g0, {})
        for ci in range(1, nch):
            g = gtiles(ci)
            stage_a_pair(ci, 0, g)
            stage_b_half(prev[0], 0, prev[1], prev[2])
            stage_a_pair(ci, 1, g)
            stage_b_half(prev[0], 1, prev[1], prev[2])
            prev = (ci, g, {})
        stage_b_fp_outer(prev[0], prev[1], prev[2])

    nc.compile()
    return nc


LAST_RESULTS = None  # BassKernelResults of the most recent run (for test harness)


def kernel(x, Wg, bg, W1, W2, W3):
    global LAST_RESULTS
    import ml_dtypes
    from concourse.bass_utils import run_bass_kernel_spmd

    f8np = ml_dtypes.float8_e4m3

    x = np.asarray(x)
    Wg, bg = np.asarray(Wg), np.asarray(bg)
    W1, W2, W3 = np.asarray(W1), np.asarray(W2), np.asarray(W3)
    B, S, d = x.shape
    T = B * S
    assert d == D and Wg.shape == (E, D)

    xf = np.ascontiguousarray(x.reshape(T, D))

    # ---- host gate + top-1 routing (fp64: exact vs any fp32 backend) ----
    gate = xf.astype(np.float64) @ Wg.astype(np.float64).T + bg.astype(np.float64)
    eid = np.argmax(gate, axis=1)
    counts = np.bincount(eid, minlength=E)
    order = np.argsort(eid, kind="stable")
    offs = np.concatenate(([0], np.cumsum(counts)))

    C = max(MIN_C, 2 * int(-(-counts.max() // 2)))
    if C not in _cache:
        _cache[C] = _build(C)
    nc = _cache[C]

    def q8(a):
        return a.astype(f8np)

    def hi_lo(a, s):
        hi = q8(a * s)
        lo = q8((a * s - hi.astype(np.float32)) * 16.0)
        return hi, lo

    def pack(a, npair):
        # [R, cols] with R = npair*2*128 -> [npair, P, 2, cols]
        cols = a.shape[1]
        return a.reshape(npair, 2, P, cols).transpose(0, 2, 1, 3)

    # ---- per-core inputs (dispatch) ----
    in_maps = []
    tok_lists = []
    for e in range(E):
        toks = order[offs[e]:offs[e + 1]]
        tok_lists.append(toks)
        ce = len(toks)

        xT = np.zeros((D, C), dtype=np.float32)
        if ce:
            xT[:, :ce] = xf[toks].T
        xhi, xlo = hi_lo(xT, SX)
        xlos = q8(xlo.astype(np.float32) / 16.0)
        xhis = q8(xhi.astype(np.float32) / 16.0)
        xq = np.stack([pack(v, KP) for v in (xhi, xlos, xhis)], axis=2)

        w1T = np.zeros((D, FP), dtype=np.float32)
        w1T[:, :F] = W1[e].T
        w1hi, w1lo = hi_lo(w1T, SW1)
        w2T = np.zeros((D, FP), dtype=np.float32)
        w2T[:, :F] = W2[e].T
        w2hi, w2lo = hi_lo(w2T, SW2)
        w12 = np.stack([pack(v, KP) for v in (w1hi, w2hi, w1lo, w2lo)], axis=2)

        w3T = np.zeros((FP, D), dtype=np.float32)
        w3T[:F, :] = W3[e].T
        w3hi, w3lo = hi_lo(w3T, SW3)
        w3ls = q8(w3lo.astype(np.float32) / 16.0)
        w3 = np.stack([pack(v, FPAIR) for v in (w3hi, w3ls)], axis=2)

        in_maps.append({
            "xq": np.ascontiguousarray(xq),
            "w12": np.ascontiguousarray(w12),
            "w3": np.ascontiguousarray(w3),
        })

    res = run_bass_kernel_spmd(nc, in_maps, list(range(E)))
    LAST_RESULTS = res

    # ---- combine: scatter outputs back to token order ----
    y = np.empty((T, D), dtype=np.float32)
    for e in range(E):
        toks = tok_lists[e]
        if len(toks):
            yT = res.results[e]["yt"].reshape(D, C)
            y[toks] = yT[:, :len(toks)].T.astype(np.float32)
    return y.reshape(B, S, d)
